# revision 6
# baseline (speedup 1.0000x reference)
"""AttentionPool2d (masked, 100-mask sparse attention) on 8 TRN2 NeuronCores.

Algorithm notes
---------------
The reference returns out[0] -- only the cls/mean query token. So per (b, h)
we only need scores0[m] = q0 . k[m], the 100-mask softmax over keys, the sum
over masks, and one weighted sum over v. Per-core sharding is by head:
core c owns heads {2c, 2c+1} = E-channels [128c, 128c+128). q/k/v weight
rows and c_w columns are sharded accordingly (weights fully partitioned,
no replication); x / pos_emb / (subsampled) mask are replicated.

v2 design (vs the 33.5us v1):
- All inputs in ONE bf16 blob [128, 9363] per core; no collective: each
  core writes a [4, 512] bf16 partial c-proj (rows 0/1 = b0/b1 for out
  cols 0-511, rows 2/3 for cols 512-1023); host sums the 8 partials.
- DMA: 7 input DMAs on the two HWDGE queues only (scalar+sync), in
  consumer order; gpsimd does NO dma so the Pool engine is free for
  elementwise work (v1 lost it to an 8us SWDGE DRAIN block). Fewer,
  bigger chunks amortize the ~1-2us per-DMA completion latency, and the
  NEFF epilogue's per-DMA-queue semaphore-zero loop (~0.5us per DMA
  instruction, measured) shrinks.
- PE pstate warm-up: ~6 dummy 512-col matmuls (ones x ones) issued at
  t=0 keep the tensor engine continuously busy through the DMA fill so
  it ramps to full clock (2.4GHz) before the real K/V matmuls; an idle
  PE runs at 0.65-1.2GHz (cost model + HW scan: >3us continuous busy =>
  max speed).
- Per-et xs assembly fuses the mean token: the x+pos add runs as
  scalar_tensor_tensor with accum_out, so the token-mean comes from the
  add's row-sum (minus a per-et pos row-sum) instead of dedicated
  reduces. Split: gpsimd does the pos reduce + b1 add, vector does the
  b0 add + mean fix-up columns.
- Tail split by engine respecting PSUM access rules (gpsimd has NO PSUM
  port): vector takes SM muls/normalize/attn_out, scalar takes the exps
  (+k_sb b1/v_sb via Identity-activation bias adds), gpsimd takes the
  SBUF-only q0r/RREP/A0r ops. A tiny dummy Exp right after the mask
  sigmoids forces both activation-table loads into the idle window.
- c-proj flipped: lhsT = A0r [128, 2] (one cheap LDWEIGHTS), rhs = cwt
  [128, 512] halves -> out [2, 512] at PSUM rows 100-103; c_b/8 is added
  in the PSUM->SBUF copy (stt) against a host-packed [4, 512] c_b tile
  living in blob rows 100-103 of the mask region. Replaces v1's 8
  matmuls + 8 ldweights + transposed [128, 16] output.
- Token axis padded 197 -> 198 per block: col 0 = mean token, col 197 =
  zero pad. K pad = kb (masked out, mask pad col = 0 so exp(0)=1 and the
  row sum gets a "-1" correction), V pad excluded by restricting the
  final weighted sum to 197 cols.
- fp8 was evaluated and is numerically DEAD here: e4m3's ~4% rms
  per-element error passes through the random-sign weighted sums
  essentially unattenuated (softmax does not average it away), blowing
  the 2e-2 budget. bf16 (~4e-3 end-to-end) is the floor.
- Known hardware landmines (verified by device faults in the v1
  session; sim passes all of them): Exp activation with accum_out
  reading a bf16 input tile, and 3-D rearrange/broadcast_to APs on DVE
  ops, both fault the NeuronCore. Keep sm tiles f32 and keep per-(b,et)
  vector ops explicit 2-D.
"""
import os

import numpy as np

B = 2
H = 16
E = 1024
SP = 14
S = SP * SP          # 196
NM = 100
L = S + 1            # 197
TB = 198             # padded token block
NET = 8              # e-tiles of 128
HD = 64
NCORES = 8
SCALE = HD ** -0.5   # 0.125
INV_S = 1.0 / S

ETCOLS = 384 + 3 * TB          # 978 cols per et block: wk|wv|wq|x_b0|x_b1|pos
TAIL0 = NET * ETCOLS           # 7824
# tail layout: kb vb qb (3) | maskcb (512: rows 0-99 = mask b0|b1,
#   rows 100-103 = c_b [4,512]) | cwt (1024)
C_KB = TAIL0
C_VB = C_KB + 1
C_QB = C_KB + 2
C_MASK = C_KB + 3
C_CWT = C_MASK + 2 * S
NCOLS = C_CWT + E              # 9243

_STATE = {}


def _build():
    import concourse.bass as bass
    import concourse.mybir as mybir
    from concourse import bacc, tile

    F32 = mybir.dt.float32
    BF16 = mybir.dt.bfloat16
    AF = mybir.ActivationFunctionType
    ALU = mybir.AluOpType
    AX = mybir.AxisListType

    nc = bacc.Bacc("TRN2", target_bir_lowering=False, debug=False,
                   num_devices=NCORES)

    blob_ap = nc.dram_tensor("blob", [128, NCOLS], BF16,
                             kind="ExternalInput").ap()
    cb_ap = nc.dram_tensor("cb", [34, 512], BF16, kind="ExternalInput").ap()
    out_ap = nc.dram_tensor("out", [34, 512], BF16, kind="ExternalOutput").ap()

    with tile.TileContext(nc) as tc:
        with (
            tc.tile_pool(name="sb", bufs=1) as sb,
            tc.tile_pool(name="ps", bufs=1, space="PSUM") as ps,
        ):
            BL = sb.tile([128, NCOLS], BF16, tag="blob")
            ones = sb.tile([128, 512], BF16, tag="ones")
            nc.gpsimd.memset(ones[:], 1.0)

            # ---- input DMA: 2 HWDGE queues, consumer-ordered ----
            chunks = [
                (nc.scalar, 0, ETCOLS),                  # et0
                (nc.sync, TAIL0, C_CWT),                 # biases+mask+cb
                (nc.sync, ETCOLS, 2 * ETCOLS),           # et1
                (nc.scalar, 2 * ETCOLS, 4 * ETCOLS),     # et2-3
                (nc.sync, 4 * ETCOLS, 6 * ETCOLS),       # et4-5
                (nc.scalar, 6 * ETCOLS, TAIL0),          # et6-7
                (nc.sync, C_CWT, NCOLS),                 # cwt
            ]
            cbt = sb.tile([34, 512], BF16, tag="cbt")
            nc.sync.dma_start(cbt[:], cb_ap[:])
            for eng, a, b2 in chunks:
                eng.dma_start(BL[:, a:b2], blob_ap[:, a:b2])

            # ---- PE pstate warm-up: keep the tensor engine busy through
            # the DMA fill so real matmuls run at full clock ----
            o_ps = ps.tile([128, 512], F32, tag="ops")
            for d in range(6):
                nc.tensor.matmul(o_ps[:, 0:512], ones[:, 0:128],
                                 ones[:, 0:512], start=True, stop=True)

            # scalar-column operands must be f32: copy biases out of the blob
            biasf = sb.tile([128, 3], F32, tag="biasf")
            nc.vector.tensor_copy(biasf[:], BL[:, C_KB:C_KB + 3])
            kb_col = biasf[:, 0:1]
            vb_col = biasf[:, 1:2]
            qb_col = biasf[:, 2:3]

            # c_b/8 tile for the final bias add (rows 0,1,32,33 live)
            cb8 = sb.tile([34, 512], BF16, tag="cb8")
            nc.vector.tensor_scalar_mul(cb8[:], cbt[:], 1.0 / NCORES)

            # ---- masks: [1 | sigmoid(196) | 0] per b ----
            msb = sb.tile([NM, 2 * TB], BF16, tag="msb")
            nc.gpsimd.memset(msb[:, 0:TB + 1:TB], 1.0)       # cols 0, 198
            nc.gpsimd.memset(msb[:, L:2 * TB:TB], 0.0)       # cols 197, 395
            for b in range(B):
                nc.scalar.activation(
                    msb[:, b * TB + 1: b * TB + L],
                    BL[0:NM, C_MASK + b * S: C_MASK + (b + 1) * S],
                    AF.Sigmoid)
            # tiny dummy Exp: forces the exp act-table load into the idle
            # window instead of the tail's critical path
            dexp = sb.tile([NM, 1], F32, tag="dexp")
            nc.scalar.activation(dexp[:], biasf[0:NM, 0:1], AF.Exp)

            # ---- per-et: xs assembly (x+pos with fused mean), K/V/q0 ----
            K_ps = ps.tile([128, 512], F32, tag="kps")
            V_ps = ps.tile([128, 512], F32, tag="vps")
            q0_ps = ps.tile([128, 512], F32, tag="q0ps")
            for et in range(NET):
                base = et * ETCOLS
                xs = sb.tile([128, 2 * TB], BF16, tag=f"xs{et}")
                ms = sb.tile([128, 2], F32, tag=f"ms{et}")
                pos = BL[:, base + 384 + 2 * TB: base + 384 + 3 * TB]
                # gpsimd: both x+pos adds (Pool has no TensorScalarPtr or
                # free-axis reduce); vector: token means + mean columns
                for b in range(B):
                    xb = BL[:, base + 384 + b * TB: base + 384 + (b + 1) * TB]
                    nc.vector.reduce_sum(ms[:, b:b + 1], xb[:, 1:L], axis=AX.X)
                    nc.gpsimd.tensor_add(
                        xs[:, b * TB + 1:(b + 1) * TB], xb[:, 1:TB],
                        pos[:, 1:TB])
                    # col 0 = mean + pos[0]
                    nc.vector.scalar_tensor_tensor(
                        xs[:, b * TB: b * TB + 1], ms[:, b:b + 1], INV_S,
                        pos[:, 0:1], op0=ALU.mult, op1=ALU.add)
                st = dict(start=(et == 0), stop=(et == NET - 1))
                nc.tensor.matmul(q0_ps[:, 0:2], BL[:, base + 256:base + 384],
                                 xs[:, 0:2 * TB:TB], **st)
                nc.tensor.matmul(K_ps[:, 0:2 * TB], BL[:, base:base + 128],
                                 xs[:], **st)
                nc.tensor.matmul(V_ps[:, 0:2 * TB], BL[:, base + 128:base + 256],
                                 xs[:], **st)

            # q0 = (q0_raw + qb) * 0.125
            q0_sb = sb.tile([128, 2], F32, tag="q0sb")
            nc.vector.tensor_scalar(q0_sb[:], q0_ps[:, 0:2], qb_col, SCALE,
                                    op0=ALU.add, op1=ALU.mult)
            # q0 replicated over 100 mask-columns (lhsT for scores matmul)
            q0r = sb.tile([128, 2 * NM], BF16, tag="q0r")
            k_sb = sb.tile([128, 2 * TB], BF16, tag="ksb")
            v_sb = sb.tile([128, 2 * TB], BF16, tag="vsb")
            for b in range(B):
                nc.vector.tensor_scalar_mul(q0r[:, b * NM:(b + 1) * NM],
                                            ones[:, 0:NM], q0_sb[:, b:b + 1])
            nc.vector.tensor_scalar_add(k_sb[:, 0:TB], K_ps[:, 0:TB], kb_col)
            nc.scalar.activation(k_sb[:, TB:2 * TB], K_ps[:, TB:2 * TB],
                                 AF.Identity, bias=kb_col)
            nc.scalar.activation(v_sb[:], V_ps[:, 0:2 * TB],
                                 AF.Identity, bias=vb_col)

            # ---- attention tail, b0/b1 interleaved ----
            A0 = sb.tile([128, 2], F32, tag="a0")
            S_ps = [ps.tile([NM, 512], F32, tag=f"sps{b}", name=f"sps{b}") for b in range(B)]
            SM = [sb.tile([NM, 2 * TB], F32, tag=f"sm{b}", name=f"sm{b}") for b in range(B)]
            E_sb = [sb.tile([NM, 2 * TB], BF16, tag=f"e{b}", name=f"e{b}") for b in range(B)]
            RS = [sb.tile([NM, 2], F32, tag=f"rs{b}", name=f"rs{b}") for b in range(B)]
            RC = [sb.tile([NM, 2], F32, tag=f"rcol{b}", name=f"rcol{b}") for b in range(B)]
            RS1 = [sb.tile([NM, 2], F32, tag=f"rs1{b}", name=f"rs1{b}") for b in range(B)]
            RREP = [sb.tile([NM, 128], BF16, tag=f"rrep{b}", name=f"rrep{b}") for b in range(B)]
            W_ps = [ps.tile([128, 512], F32, tag=f"wps{b}", name=f"wps{b}") for b in range(B)]

            def scores_sm(b):
                for h in range(2):
                    sl = slice(h * HD, (h + 1) * HD)
                    nc.tensor.matmul(S_ps[b][:, h * TB:(h + 1) * TB],
                                     q0r[sl, b * NM:(b + 1) * NM],
                                     k_sb[sl, b * TB:(b + 1) * TB],
                                     start=True, stop=True)
                    nc.vector.tensor_mul(SM[b][:, h * TB:(h + 1) * TB],
                                         S_ps[b][:, h * TB:(h + 1) * TB],
                                         msb[:, b * TB:(b + 1) * TB])

            def exps(b):
                # pad col of sm is 0 -> exp=1; row sum corrected by -1
                for h in range(2):
                    nc.scalar.activation(E_sb[b][:, h * TB:(h + 1) * TB],
                                         SM[b][:, h * TB:(h + 1) * TB], AF.Exp,
                                         accum_out=RS[b][:, h:h + 1])

            def normalize(b):
                nc.vector.tensor_scalar_add(RS1[b][:], RS[b][:], -1.0)
                nc.vector.reciprocal(RC[b][:], RS1[b][:])
                for h in range(2):
                    nc.vector.tensor_scalar_mul(
                        RREP[b][:, h * HD:(h + 1) * HD], ones[0:NM, 0:HD],
                        RC[b][:, h:h + 1])

            def weighted_v(b):
                # one matmul per head, output at partition base h*64 so both
                # heads' weight rows line up with V's channel layout
                for h in range(2):
                    nc.tensor.matmul(W_ps[b][h * HD:(h + 1) * HD, 0:TB],
                                     RREP[b][:, h * HD:(h + 1) * HD],
                                     E_sb[b][:, h * TB:(h + 1) * TB],
                                     start=True, stop=True)

            def attn_out(b):
                t = sb.tile([128, L], BF16, tag=f"t{b}", name=f"t{b}")
                # attn0[c] = sum_d w[d] * v[c, d] over the 197 real cols
                nc.vector.scalar_tensor_tensor(
                    t[:], W_ps[b][:, 0:L], 1.0,
                    v_sb[:, b * TB: b * TB + L],
                    op0=ALU.mult, op1=ALU.mult,
                    accum_out=A0[:, b:b + 1])

            scores_sm(0)
            exps(0)
            scores_sm(1)
            exps(1)
            normalize(0)
            weighted_v(0)
            normalize(1)
            attn_out(0)
            weighted_v(1)
            attn_out(1)

            # ---- c-proj, flipped: out[2c+b, k] = sum_e A0[e,b] cwt[e, 512c+k]
            A0r = sb.tile([128, 2], BF16, tag="a0r")
            nc.gpsimd.tensor_copy(A0r[:], A0[:])
            for c in range(2):
                nc.tensor.matmul(o_ps[32 * c:32 * c + 2, 0:512],
                                 A0r[:],
                                 BL[:, C_CWT + 512 * c: C_CWT + 512 * (c + 1)],
                                 start=True, stop=True)
            o_sb = sb.tile([34, 512], BF16, tag="osb")
            nc.vector.scalar_tensor_tensor(
                o_sb[:], o_ps[0:34, :], 1.0, cb8[:],
                op0=ALU.mult, op1=ALU.add)
            nc.scalar.dma_start(out_ap[:], o_sb[:])

    nc.compile()
    return nc


def _get_nc():
    if "nc" not in _STATE:
        _STATE["nc"] = _build()
    return _STATE["nc"]


def _make_in_maps(inputs):
    """Host-side packing: pure layout/dtype movement into one blob per core."""
    import ml_dtypes

    x = np.asarray(inputs["x"], np.float32).reshape(B, E, S)
    mask_feature = np.asarray(inputs["mask_feature"], np.float32)
    pos_t = np.ascontiguousarray(np.asarray(inputs["pos_emb"], np.float32).T)
    q_w = np.asarray(inputs["q_w"], np.float32)
    q_b = np.asarray(inputs["q_b"], np.float32)
    k_w = np.asarray(inputs["k_w"], np.float32)
    k_b = np.asarray(inputs["k_b"], np.float32)
    v_w = np.asarray(inputs["v_w"], np.float32)
    v_b = np.asarray(inputs["v_b"], np.float32)
    c_w = np.asarray(inputs["c_w"], np.float32)
    c_b = np.asarray(inputs["c_b"], np.float32)

    mask12 = mask_feature[:, :, ::8, ::8].reshape(B, NM, S)

    in_maps = []
    for c in range(NCORES):
        ch = slice(c * 128, (c + 1) * 128)
        blob = np.zeros((128, NCOLS), np.float32)
        for et in range(NET):
            base = et * ETCOLS
            eslc = slice(et * 128, (et + 1) * 128)
            blob[:, base:base + 128] = k_w[ch, eslc].T
            blob[:, base + 128:base + 256] = v_w[ch, eslc].T
            blob[:, base + 256:base + 384] = q_w[ch, eslc].T
            for b in range(B):
                blob[:, base + 384 + b * TB + 1: base + 384 + b * TB + L] = \
                    x[b, eslc]
            blob[:, base + 384 + 2 * TB: base + 384 + 2 * TB + L] = \
                pos_t[eslc]
        blob[:, C_CWT:C_CWT + E] = c_w[:, ch].T
        blob[:, C_KB] = k_b[ch]
        blob[:, C_VB] = v_b[ch]
        blob[:, C_QB] = q_b[ch]
        blob[0:NM, C_MASK:C_MASK + S] = mask12[0]
        blob[0:NM, C_MASK + S:C_MASK + 2 * S] = mask12[1]
        # c_b rows 0/1 = c_b[0:512] (b0/b1), rows 32/33 = c_b[512:1024]
        cb = np.zeros((34, 512), np.float32)
        cb[0:2] = c_b[0:512]
        cb[32:34] = c_b[512:1024]
        in_maps.append({"blob": blob.astype(ml_dtypes.bfloat16),
                        "cb": cb.astype(ml_dtypes.bfloat16)})
    return in_maps


def _unshard(parts):
    """Sum per-core partial outputs [34, 512] -> [B, E]."""
    R = np.zeros((34, 512), np.float64)
    for p in parts:
        R += np.asarray(p, np.float32)
    return np.ascontiguousarray(
        np.concatenate([R[0:2], R[32:34]], axis=1)).astype(np.float32)


def kernel(**inputs):
    in_maps = _make_in_maps(inputs)

    from concourse.bass_utils import run_bass_kernel_spmd

    nc = _get_nc()
    trace = bool(int(os.environ.get("KERNEL_TRACE", "0")))
    if trace:
        try:
            import ntff_hook
            ntff_hook.install()
        except Exception:
            pass
    res = run_bass_kernel_spmd(nc, in_maps, list(range(NCORES)), trace=trace)
    _STATE["last_exec_ns"] = res.exec_time_ns
    _STATE["last_results"] = res
    return _unshard([res.results[c]["out"] for c in range(NCORES)])


# revision 8
# speedup vs baseline: 1.0082x; 1.0082x over previous
"""AttentionPool2d (masked, 100-mask sparse attention) on 8 TRN2 NeuronCores.

Algorithm notes
---------------
The reference returns out[0] -- only the cls/mean query token. So per (b, h)
we only need scores0[m] = q0 . k[m], the 100-mask softmax over keys, the sum
over masks, and one weighted sum over v. Per-core sharding is by head:
core c owns heads {2c, 2c+1} = E-channels [128c, 128c+128). q/k/v weight
rows and c_w columns are sharded accordingly (weights fully partitioned,
no replication); x / pos_emb / (subsampled) mask are replicated.

v2 design (vs the 33.5us v1):
- All inputs in ONE bf16 blob [128, 9363] per core; no collective: each
  core writes a [4, 512] bf16 partial c-proj (rows 0/1 = b0/b1 for out
  cols 0-511, rows 2/3 for cols 512-1023); host sums the 8 partials.
- DMA: 7 input DMAs on the two HWDGE queues only (scalar+sync), in
  consumer order; gpsimd does NO dma so the Pool engine is free for
  elementwise work (v1 lost it to an 8us SWDGE DRAIN block). Fewer,
  bigger chunks amortize the ~1-2us per-DMA completion latency, and the
  NEFF epilogue's per-DMA-queue semaphore-zero loop (~0.5us per DMA
  instruction, measured) shrinks.
- PE pstate warm-up: ~6 dummy 512-col matmuls (ones x ones) issued at
  t=0 keep the tensor engine continuously busy through the DMA fill so
  it ramps to full clock (2.4GHz) before the real K/V matmuls; an idle
  PE runs at 0.65-1.2GHz (cost model + HW scan: >3us continuous busy =>
  max speed).
- Per-et xs assembly fuses the mean token: the x+pos add runs as
  scalar_tensor_tensor with accum_out, so the token-mean comes from the
  add's row-sum (minus a per-et pos row-sum) instead of dedicated
  reduces. Split: gpsimd does the pos reduce + b1 add, vector does the
  b0 add + mean fix-up columns.
- Tail split by engine respecting PSUM access rules (gpsimd has NO PSUM
  port): vector takes SM muls/normalize/attn_out, scalar takes the exps
  (+k_sb b1/v_sb via Identity-activation bias adds), gpsimd takes the
  SBUF-only q0r/RREP/A0r ops. A tiny dummy Exp right after the mask
  sigmoids forces both activation-table loads into the idle window.
- c-proj flipped: lhsT = A0r [128, 2] (one cheap LDWEIGHTS), rhs = cwt
  [128, 512] halves -> out [2, 512] at PSUM rows 100-103; c_b/8 is added
  in the PSUM->SBUF copy (stt) against a host-packed [4, 512] c_b tile
  living in blob rows 100-103 of the mask region. Replaces v1's 8
  matmuls + 8 ldweights + transposed [128, 16] output.
- Token axis padded 197 -> 198 per block: col 0 = mean token, col 197 =
  zero pad. K pad = kb (masked out, mask pad col = 0 so exp(0)=1 and the
  row sum gets a "-1" correction), V pad excluded by restricting the
  final weighted sum to 197 cols.
- fp8 was evaluated and is numerically DEAD here: e4m3's ~4% rms
  per-element error passes through the random-sign weighted sums
  essentially unattenuated (softmax does not average it away), blowing
  the 2e-2 budget. bf16 (~4e-3 end-to-end) is the floor.
- Known hardware landmines (verified by device faults in the v1
  session; sim passes all of them): Exp activation with accum_out
  reading a bf16 input tile, and 3-D rearrange/broadcast_to APs on DVE
  ops, both fault the NeuronCore. Keep sm tiles f32 and keep per-(b,et)
  vector ops explicit 2-D.
"""
import os

import numpy as np

B = 2
H = 16
E = 1024
SP = 14
S = SP * SP          # 196
NM = 100
L = S + 1            # 197
TB = 198             # padded token block
NET = 8              # e-tiles of 128
HD = 64
NCORES = 8
SCALE = HD ** -0.5   # 0.125
INV_S = 1.0 / S

ETCOLS = 384 + 3 * TB          # 978 cols per et block: wk|wv|wq|x_b0|x_b1|pos
TAIL0 = NET * ETCOLS           # 7824
# tail layout: kb vb qb (3) | maskcb (512: rows 0-99 = mask b0|b1,
#   rows 100-103 = c_b [4,512]) | cwt (1024)
C_KB = TAIL0
C_VB = C_KB + 1
C_QB = C_KB + 2
C_MASK = C_KB + 3
C_CWT = C_MASK + 2 * S
NCOLS = C_CWT + E              # 9243

_STATE = {}


def _build():
    import concourse.bass as bass
    import concourse.mybir as mybir
    from concourse import bacc, tile

    F32 = mybir.dt.float32
    BF16 = mybir.dt.bfloat16
    AF = mybir.ActivationFunctionType
    ALU = mybir.AluOpType
    AX = mybir.AxisListType

    nc = bacc.Bacc("TRN2", target_bir_lowering=False, debug=False,
                   num_devices=NCORES)

    blob_ap = nc.dram_tensor("blob", [128, NCOLS], BF16,
                             kind="ExternalInput").ap()
    cb_ap = nc.dram_tensor("cb", [34, 512], BF16, kind="ExternalInput").ap()
    out_ap = nc.dram_tensor("out", [34, 512], BF16, kind="ExternalOutput").ap()

    with tile.TileContext(nc) as tc:
        with (
            tc.tile_pool(name="sb", bufs=1) as sb,
            tc.tile_pool(name="ps", bufs=1, space="PSUM") as ps,
        ):
            BL = sb.tile([128, NCOLS], BF16, tag="blob")
            ones = sb.tile([128, 512], BF16, tag="ones")
            nc.gpsimd.memset(ones[:], 1.0)

            # ---- input DMA: 2 HWDGE queues, consumer-ordered ----
            cbt = sb.tile([34, 512], BF16, tag="cbt")
            nc.sync.dma_start(cbt[:], cb_ap[:])
            chunks = [
                (nc.scalar, 0, ETCOLS),                  # et0
                (nc.sync, ETCOLS, 2 * ETCOLS),           # et1
                (nc.scalar, 2 * ETCOLS, 4 * ETCOLS),     # et2-3
                (nc.sync, TAIL0, C_CWT),                 # biases+mask
                (nc.sync, 4 * ETCOLS, 6 * ETCOLS),       # et4-5
                (nc.scalar, 6 * ETCOLS, TAIL0),          # et6-7
                (nc.sync, C_CWT, NCOLS),                 # cwt
            ]
            for eng, a, b2 in chunks:
                eng.dma_start(BL[:, a:b2], blob_ap[:, a:b2])

            o_ps = ps.tile([128, 512], F32, tag="ops")

            # scalar-column operands must be f32: copy biases out of the blob
            biasf = sb.tile([128, 3], F32, tag="biasf")
            nc.vector.tensor_copy(biasf[:], BL[:, C_KB:C_KB + 3])
            kb_col = biasf[:, 0:1]
            vb_col = biasf[:, 1:2]
            qb_col = biasf[:, 2:3]

            # c_b/8 preloaded into the c-proj PSUM rows; matmuls accumulate
            nc.vector.tensor_scalar_mul(o_ps[0:2, :], cbt[0:2, :],
                                        1.0 / NCORES)
            nc.vector.tensor_scalar_mul(o_ps[32:34, :], cbt[32:34, :],
                                        1.0 / NCORES)

            # ---- masks: [1 | sigmoid(196) | 0] per b ----
            msb = sb.tile([NM, 2 * TB], BF16, tag="msb")
            nc.gpsimd.memset(msb[:, 0:TB + 1:TB], 1.0)       # cols 0, 198
            nc.gpsimd.memset(msb[:, L:2 * TB:TB], 0.0)       # cols 197, 395
            for b in range(B):
                nc.scalar.activation(
                    msb[:, b * TB + 1: b * TB + L],
                    BL[0:NM, C_MASK + b * S: C_MASK + (b + 1) * S],
                    AF.Sigmoid)
            # tiny dummy Exp: forces the exp act-table load into the idle
            # window instead of the tail's critical path
            dexp = sb.tile([NM, 1], F32, tag="dexp")
            nc.scalar.activation(dexp[:], biasf[0:NM, 0:1], AF.Exp)

            # ---- per-et: xs assembly (x+pos with fused mean), K/V/q0 ----
            K_ps = ps.tile([128, 512], F32, tag="kps")
            V_ps = ps.tile([128, 512], F32, tag="vps")
            q0_ps = ps.tile([128, 512], F32, tag="q0ps")
            for et in range(NET):
                base = et * ETCOLS
                xs = sb.tile([128, 2 * TB], BF16, tag=f"xs{et}")
                ms = sb.tile([128, 2], F32, tag=f"ms{et}")
                pos = BL[:, base + 384 + 2 * TB: base + 384 + 3 * TB]
                for b in range(B):
                    xb = BL[:, base + 384 + b * TB: base + 384 + (b + 1) * TB]
                    nc.vector.reduce_sum(ms[:, b:b + 1], xb[:, 1:L], axis=AX.X)
                    nc.vector.tensor_add(
                        xs[:, b * TB + 1:(b + 1) * TB], xb[:, 1:TB],
                        pos[:, 1:TB])
                    # col 0 = mean + pos[0]
                    nc.vector.scalar_tensor_tensor(
                        xs[:, b * TB: b * TB + 1], ms[:, b:b + 1], INV_S,
                        pos[:, 0:1], op0=ALU.mult, op1=ALU.add)
                st = dict(start=(et == 0), stop=(et == NET - 1))
                nc.tensor.matmul(q0_ps[:, 0:2], BL[:, base + 256:base + 384],
                                 xs[:, 0:2 * TB:TB], **st)
                nc.tensor.matmul(K_ps[:, 0:2 * TB], BL[:, base:base + 128],
                                 xs[:], **st)
                nc.tensor.matmul(V_ps[:, 0:2 * TB], BL[:, base + 128:base + 256],
                                 xs[:], **st)

            # q0 = (q0_raw + qb) * 0.125
            q0_sb = sb.tile([128, 2], F32, tag="q0sb")
            nc.vector.tensor_scalar(q0_sb[:], q0_ps[:, 0:2], qb_col, SCALE,
                                    op0=ALU.add, op1=ALU.mult)
            # q0 replicated over 100 mask-columns (lhsT for scores matmul)
            q0r = sb.tile([128, 2 * NM], BF16, tag="q0r")
            k_sb = sb.tile([128, 2 * TB], BF16, tag="ksb")
            v_sb = sb.tile([128, 2 * TB], BF16, tag="vsb")
            for b in range(B):
                nc.vector.tensor_scalar_mul(q0r[:, b * NM:(b + 1) * NM],
                                            ones[:, 0:NM], q0_sb[:, b:b + 1])
            for b in range(B):
                nc.vector.tensor_scalar_add(k_sb[:, b * TB:(b + 1) * TB],
                                            K_ps[:, b * TB:(b + 1) * TB],
                                            kb_col)

            # ---- attention tail, b0/b1 interleaved ----
            A0 = sb.tile([128, 2], F32, tag="a0")
            S_ps = [ps.tile([NM, 512], F32, tag=f"sps{b}", name=f"sps{b}") for b in range(B)]
            SM = [sb.tile([NM, 2 * TB], F32, tag=f"sm{b}", name=f"sm{b}") for b in range(B)]
            E_sb = [sb.tile([NM, 2 * TB], BF16, tag=f"e{b}", name=f"e{b}") for b in range(B)]
            RS = [sb.tile([NM, 2], F32, tag=f"rs{b}", name=f"rs{b}") for b in range(B)]
            RC = [sb.tile([NM, 2], F32, tag=f"rcol{b}", name=f"rcol{b}") for b in range(B)]
            RS1 = [sb.tile([NM, 2], F32, tag=f"rs1{b}", name=f"rs1{b}") for b in range(B)]
            RREP = [sb.tile([NM, 128], BF16, tag=f"rrep{b}", name=f"rrep{b}") for b in range(B)]
            W_ps = [ps.tile([128, 512], F32, tag=f"wps{b}", name=f"wps{b}") for b in range(B)]

            def scores_sm(b):
                for h in range(2):
                    sl = slice(h * HD, (h + 1) * HD)
                    nc.tensor.matmul(S_ps[b][:, h * TB:(h + 1) * TB],
                                     q0r[sl, b * NM:(b + 1) * NM],
                                     k_sb[sl, b * TB:(b + 1) * TB],
                                     start=True, stop=True)
                    nc.vector.tensor_mul(SM[b][:, h * TB:(h + 1) * TB],
                                         S_ps[b][:, h * TB:(h + 1) * TB],
                                         msb[:, b * TB:(b + 1) * TB])

            def exps(b):
                # pad col of sm is 0 -> exp=1; row sum corrected by -1
                for h in range(2):
                    nc.scalar.activation(E_sb[b][:, h * TB:(h + 1) * TB],
                                         SM[b][:, h * TB:(h + 1) * TB], AF.Exp,
                                         accum_out=RS[b][:, h:h + 1])

            def normalize(b):
                nc.vector.tensor_scalar_add(RS1[b][:], RS[b][:], -1.0)
                nc.vector.reciprocal(RC[b][:], RS1[b][:])
                for h in range(2):
                    nc.vector.tensor_scalar_mul(
                        RREP[b][:, h * HD:(h + 1) * HD], ones[0:NM, 0:HD],
                        RC[b][:, h:h + 1])

            def weighted_v(b):
                # one matmul per head, output at partition base h*64 so both
                # heads' weight rows line up with V's channel layout
                for h in range(2):
                    nc.tensor.matmul(W_ps[b][h * HD:(h + 1) * HD, 0:TB],
                                     RREP[b][:, h * HD:(h + 1) * HD],
                                     E_sb[b][:, h * TB:(h + 1) * TB],
                                     start=True, stop=True)

            def attn_out(b):
                t = sb.tile([128, L], BF16, tag=f"t{b}", name=f"t{b}")
                # attn0[c] = sum_d w[d] * v[c, d] over the 197 real cols
                nc.vector.scalar_tensor_tensor(
                    t[:], W_ps[b][:, 0:L], 1.0,
                    v_sb[:, b * TB: b * TB + L],
                    op0=ALU.mult, op1=ALU.mult,
                    accum_out=A0[:, b:b + 1])

            scores_sm(0)
            exps(0)
            scores_sm(1)
            exps(1)
            normalize(0)
            nc.vector.tensor_scalar_add(v_sb[:], V_ps[:, 0:2 * TB], vb_col)
            weighted_v(0)
            normalize(1)
            attn_out(0)
            weighted_v(1)
            attn_out(1)

            # ---- c-proj, flipped: out[2c+b, k] = sum_e A0[e,b] cwt[e, 512c+k]
            A0r = sb.tile([128, 2], BF16, tag="a0r")
            nc.gpsimd.tensor_copy(A0r[:], A0[:])
            for c in range(2):
                nc.tensor.matmul(o_ps[32 * c:32 * c + 2, 0:512],
                                 A0r[:],
                                 BL[:, C_CWT + 512 * c: C_CWT + 512 * (c + 1)],
                                 start=False, stop=True, skip_group_check=True)
            o_sb = sb.tile([34, 512], BF16, tag="osb")
            nc.vector.tensor_copy(o_sb[0:2, :], o_ps[0:2, :])
            nc.vector.tensor_copy(o_sb[32:34, :], o_ps[32:34, :])
            nc.scalar.dma_start(out_ap[0:2, :], o_sb[0:2, :])
            nc.sync.dma_start(out_ap[32:34, :], o_sb[32:34, :])

    nc.compile()
    return nc


def _get_nc():
    if "nc" not in _STATE:
        _STATE["nc"] = _build()
    return _STATE["nc"]


def _make_in_maps(inputs):
    """Host-side packing: pure layout/dtype movement into one blob per core."""
    import ml_dtypes

    x = np.asarray(inputs["x"], np.float32).reshape(B, E, S)
    mask_feature = np.asarray(inputs["mask_feature"], np.float32)
    pos_t = np.ascontiguousarray(np.asarray(inputs["pos_emb"], np.float32).T)
    q_w = np.asarray(inputs["q_w"], np.float32)
    q_b = np.asarray(inputs["q_b"], np.float32)
    k_w = np.asarray(inputs["k_w"], np.float32)
    k_b = np.asarray(inputs["k_b"], np.float32)
    v_w = np.asarray(inputs["v_w"], np.float32)
    v_b = np.asarray(inputs["v_b"], np.float32)
    c_w = np.asarray(inputs["c_w"], np.float32)
    c_b = np.asarray(inputs["c_b"], np.float32)

    mask12 = mask_feature[:, :, ::8, ::8].reshape(B, NM, S)

    in_maps = []
    for c in range(NCORES):
        ch = slice(c * 128, (c + 1) * 128)
        blob = np.zeros((128, NCOLS), np.float32)
        for et in range(NET):
            base = et * ETCOLS
            eslc = slice(et * 128, (et + 1) * 128)
            blob[:, base:base + 128] = k_w[ch, eslc].T
            blob[:, base + 128:base + 256] = v_w[ch, eslc].T
            blob[:, base + 256:base + 384] = q_w[ch, eslc].T
            for b in range(B):
                blob[:, base + 384 + b * TB + 1: base + 384 + b * TB + L] = \
                    x[b, eslc]
            blob[:, base + 384 + 2 * TB: base + 384 + 2 * TB + L] = \
                pos_t[eslc]
        blob[:, C_CWT:C_CWT + E] = c_w[:, ch].T
        blob[:, C_KB] = k_b[ch]
        blob[:, C_VB] = v_b[ch]
        blob[:, C_QB] = q_b[ch]
        blob[0:NM, C_MASK:C_MASK + S] = mask12[0]
        blob[0:NM, C_MASK + S:C_MASK + 2 * S] = mask12[1]
        # c_b rows 0/1 = c_b[0:512] (b0/b1), rows 32/33 = c_b[512:1024]
        cb = np.zeros((34, 512), np.float32)
        cb[0:2] = c_b[0:512]
        cb[32:34] = c_b[512:1024]
        in_maps.append({"blob": blob.astype(ml_dtypes.bfloat16),
                        "cb": cb.astype(ml_dtypes.bfloat16)})
    return in_maps


def _unshard(parts):
    """Sum per-core partial outputs [34, 512] -> [B, E]."""
    R = np.zeros((34, 512), np.float64)
    for p in parts:
        R += np.asarray(p, np.float32)
    return np.ascontiguousarray(
        np.concatenate([R[0:2], R[32:34]], axis=1)).astype(np.float32)


def kernel(**inputs):
    in_maps = _make_in_maps(inputs)

    from concourse.bass_utils import run_bass_kernel_spmd

    nc = _get_nc()
    trace = bool(int(os.environ.get("KERNEL_TRACE", "0")))
    if trace:
        try:
            import ntff_hook
            ntff_hook.install()
        except Exception:
            pass
    res = run_bass_kernel_spmd(nc, in_maps, list(range(NCORES)), trace=trace)
    _STATE["last_exec_ns"] = res.exec_time_ns
    _STATE["last_results"] = res
    return _unshard([res.results[c]["out"] for c in range(NCORES)])


# revision 9
# speedup vs baseline: 1.0405x; 1.0321x over previous
"""AttentionPool2d (masked, 100-mask sparse attention) on 8 TRN2 NeuronCores.

Algorithm notes
---------------
The reference returns out[0] -- only the cls/mean query token. So per (b, h)
we only need scores0[m] = q0 . k[m], the 100-mask softmax over keys, the sum
over masks, and one weighted sum over v. Per-core sharding is by head:
core c owns heads {2c, 2c+1} = E-channels [128c, 128c+128). q/k/v weight
rows and c_w columns are sharded accordingly (weights fully partitioned,
no replication); x / pos_emb / (subsampled) mask are replicated.

v2 design (vs the 33.5us v1):
- All inputs in ONE bf16 blob [128, 9363] per core; no collective: each
  core writes a [4, 512] bf16 partial c-proj (rows 0/1 = b0/b1 for out
  cols 0-511, rows 2/3 for cols 512-1023); host sums the 8 partials.
- DMA: 7 input DMAs on the two HWDGE queues only (scalar+sync), in
  consumer order; gpsimd does NO dma so the Pool engine is free for
  elementwise work (v1 lost it to an 8us SWDGE DRAIN block). Fewer,
  bigger chunks amortize the ~1-2us per-DMA completion latency, and the
  NEFF epilogue's per-DMA-queue semaphore-zero loop (~0.5us per DMA
  instruction, measured) shrinks.
- PE pstate warm-up: ~6 dummy 512-col matmuls (ones x ones) issued at
  t=0 keep the tensor engine continuously busy through the DMA fill so
  it ramps to full clock (2.4GHz) before the real K/V matmuls; an idle
  PE runs at 0.65-1.2GHz (cost model + HW scan: >3us continuous busy =>
  max speed).
- Per-et xs assembly fuses the mean token: the x+pos add runs as
  scalar_tensor_tensor with accum_out, so the token-mean comes from the
  add's row-sum (minus a per-et pos row-sum) instead of dedicated
  reduces. Split: gpsimd does the pos reduce + b1 add, vector does the
  b0 add + mean fix-up columns.
- Tail split by engine respecting PSUM access rules (gpsimd has NO PSUM
  port): vector takes SM muls/normalize/attn_out, scalar takes the exps
  (+k_sb b1/v_sb via Identity-activation bias adds), gpsimd takes the
  SBUF-only q0r/RREP/A0r ops. A tiny dummy Exp right after the mask
  sigmoids forces both activation-table loads into the idle window.
- c-proj flipped: lhsT = A0r [128, 2] (one cheap LDWEIGHTS), rhs = cwt
  [128, 512] halves -> out [2, 512] at PSUM rows 100-103; c_b/8 is added
  in the PSUM->SBUF copy (stt) against a host-packed [4, 512] c_b tile
  living in blob rows 100-103 of the mask region. Replaces v1's 8
  matmuls + 8 ldweights + transposed [128, 16] output.
- Token axis padded 197 -> 198 per block: col 0 = mean token, col 197 =
  zero pad. K pad = kb (masked out, mask pad col = 0 so exp(0)=1 and the
  row sum gets a "-1" correction), V pad excluded by restricting the
  final weighted sum to 197 cols.
- fp8 was evaluated and is numerically DEAD here: e4m3's ~4% rms
  per-element error passes through the random-sign weighted sums
  essentially unattenuated (softmax does not average it away), blowing
  the 2e-2 budget. bf16 (~4e-3 end-to-end) is the floor.
- Known hardware landmines (verified by device faults in the v1
  session; sim passes all of them): Exp activation with accum_out
  reading a bf16 input tile, and 3-D rearrange/broadcast_to APs on DVE
  ops, both fault the NeuronCore. Keep sm tiles f32 and keep per-(b,et)
  vector ops explicit 2-D.
"""
import os

import numpy as np

B = 2
H = 16
E = 1024
SP = 14
S = SP * SP          # 196
NM = 100
L = S + 1            # 197
TB = 198             # padded token block
NET = 8              # e-tiles of 128
HD = 64
NCORES = 8
SCALE = HD ** -0.5   # 0.125
INV_S = 1.0 / S

ETCOLS = 384 + 3 * TB          # 978 cols per et block: wk|wv|wq|x_b0|x_b1|pos
TAIL0 = NET * ETCOLS           # 7824
# tail layout: kb vb qb (3) | maskcb (512: rows 0-99 = mask b0|b1,
#   rows 100-103 = c_b [4,512]) | cwt (1024)
C_KB = TAIL0
C_VB = C_KB + 1
C_QB = C_KB + 2
C_MASK = C_KB + 3
C_P0 = C_MASK + 2 * S          # pos0pair: 16 cols, col 2e+b = pos0 of et e
C_CWT = C_P0 + 16
NCOLS = C_CWT + E              # 9259

_STATE = {}


def _build():
    import concourse.bass as bass
    import concourse.mybir as mybir
    from concourse import bacc, tile

    F32 = mybir.dt.float32
    BF16 = mybir.dt.bfloat16
    AF = mybir.ActivationFunctionType
    ALU = mybir.AluOpType
    AX = mybir.AxisListType

    nc = bacc.Bacc("TRN2", target_bir_lowering=False, debug=False,
                   num_devices=NCORES)

    blob_ap = nc.dram_tensor("blob", [128, NCOLS], BF16,
                             kind="ExternalInput").ap()
    cb_ap = nc.dram_tensor("cb", [34, 512], BF16, kind="ExternalInput").ap()
    out_ap = nc.dram_tensor("out", [34, 512], BF16, kind="ExternalOutput").ap()

    with tile.TileContext(nc) as tc:
        with (
            tc.tile_pool(name="sb", bufs=1) as sb,
            tc.tile_pool(name="ps", bufs=1, space="PSUM") as ps,
        ):
            BL = sb.tile([128, NCOLS], BF16, tag="blob")
            ones = sb.tile([128, 512], BF16, tag="ones")
            nc.gpsimd.memset(ones[:], 1.0)

            # ---- input DMA: 2 HWDGE queues, consumer-ordered ----
            cbt = sb.tile([34, 512], BF16, tag="cbt")
            nc.sync.dma_start(cbt[:], cb_ap[:])
            chunks = [
                (nc.scalar, 0, ETCOLS),                  # et0
                (nc.sync, TAIL0, C_CWT),                 # biases+mask+pos0
                (nc.sync, ETCOLS, 2 * ETCOLS),           # et1
                (nc.scalar, 2 * ETCOLS, 4 * ETCOLS),     # et2-3
                (nc.sync, 4 * ETCOLS, 6 * ETCOLS),       # et4-5
                (nc.scalar, 6 * ETCOLS, TAIL0),          # et6-7
                (nc.sync, C_CWT, NCOLS),                 # cwt
            ]
            for eng, a, b2 in chunks:
                eng.dma_start(BL[:, a:b2], blob_ap[:, a:b2])

            o_ps = ps.tile([128, 512], F32, tag="ops")

            # masks: [1 | sigmoid(196) | 0] per b (scalar engine; sits
            # before the et loop only on S/G queues, which are idle then)
            msb = sb.tile([NM, 2 * TB], BF16, tag="msb")
            nc.gpsimd.memset(msb[:, 0:TB + 1:TB], 1.0)       # cols 0, 198
            nc.gpsimd.memset(msb[:, L:2 * TB:TB], 0.0)       # cols 197, 395
            for b in range(B):
                nc.scalar.activation(
                    msb[:, b * TB + 1: b * TB + L],
                    BL[0:NM, C_MASK + b * S: C_MASK + (b + 1) * S],
                    AF.Sigmoid)
            # tiny dummy Exp: forces the exp act-table load into the idle
            # window instead of the tail's critical path
            dexp = sb.tile([NM, 1], F32, tag="dexp")
            nc.scalar.activation(dexp[:], BL[0:NM, C_KB:C_KB + 1], AF.Exp)

            # ---- per-et: xs assembly (x+pos with fused mean), K/V/q0 ----
            K_ps = ps.tile([128, 512], F32, tag="kps")
            V_ps = ps.tile([128, 512], F32, tag="vps")
            q0_ps = ps.tile([128, 512], F32, tag="q0ps")
            for et in range(NET):
                base = et * ETCOLS
                xs = sb.tile([128, 2 * TB], BF16, tag=f"xs{et}")
                ms = sb.tile([128, 2], F32, tag=f"ms{et}")
                pos = BL[:, base + 384 + 2 * TB: base + 384 + 3 * TB]
                for b in range(B):
                    xb = BL[:, base + 384 + b * TB: base + 384 + (b + 1) * TB]
                    nc.vector.reduce_sum(ms[:, b:b + 1], xb[:, 1:L], axis=AX.X)
                    nc.gpsimd.tensor_add(
                        xs[:, b * TB + 1:(b + 1) * TB], xb[:, 1:TB],
                        pos[:, 1:TB])
                # mean cols for both b in one op: host-packed pos0pair
                nc.vector.scalar_tensor_tensor(
                    xs[:, 0:2 * TB:TB], ms[:, 0:2], INV_S,
                    BL[:, C_P0 + 2 * et: C_P0 + 2 * et + 2],
                    op0=ALU.mult, op1=ALU.add)
                st = dict(start=(et == 0), stop=(et == NET - 1))
                nc.tensor.matmul(q0_ps[:, 0:2], BL[:, base + 256:base + 384],
                                 xs[:, 0:2 * TB:TB], **st)
                nc.tensor.matmul(K_ps[:, 0:2 * TB], BL[:, base:base + 128],
                                 xs[:], **st)
                nc.tensor.matmul(V_ps[:, 0:2 * TB], BL[:, base + 128:base + 256],
                                 xs[:], **st)

            # scalar-column operands must be f32: copy biases out of the blob
            biasf = sb.tile([128, 3], F32, tag="biasf")
            nc.vector.tensor_copy(biasf[:], BL[:, C_KB:C_KB + 3])
            kb_col = biasf[:, 0:1]
            vb_col = biasf[:, 1:2]
            qb_col = biasf[:, 2:3]
            # c_b/8 preloaded into the c-proj PSUM rows; matmuls accumulate
            nc.vector.tensor_scalar_mul(o_ps[0:2, :], cbt[0:2, :],
                                        1.0 / NCORES)
            nc.vector.tensor_scalar_mul(o_ps[32:34, :], cbt[32:34, :],
                                        1.0 / NCORES)

            # q0 = (q0_raw + qb) * 0.125
            q0_sb = sb.tile([128, 2], F32, tag="q0sb")
            nc.vector.tensor_scalar(q0_sb[:], q0_ps[:, 0:2], qb_col, SCALE,
                                    op0=ALU.add, op1=ALU.mult)
            # q0 replicated over 100 mask-columns (lhsT for scores matmul)
            q0r = sb.tile([128, 2 * NM], BF16, tag="q0r")
            k_sb = sb.tile([128, 2 * TB], BF16, tag="ksb")
            v_sb = sb.tile([128, 2 * TB], BF16, tag="vsb")
            for b in range(B):
                nc.vector.tensor_scalar_mul(q0r[:, b * NM:(b + 1) * NM],
                                            ones[:, 0:NM], q0_sb[:, b:b + 1])
            for b in range(B):
                nc.vector.tensor_scalar_add(k_sb[:, b * TB:(b + 1) * TB],
                                            K_ps[:, b * TB:(b + 1) * TB],
                                            kb_col)

            # ---- attention tail, b0/b1 interleaved ----
            A0 = sb.tile([128, 2], F32, tag="a0")
            S_ps = [ps.tile([NM, 512], F32, tag=f"sps{b}", name=f"sps{b}") for b in range(B)]
            SM = [sb.tile([NM, 2 * TB], F32, tag=f"sm{b}", name=f"sm{b}") for b in range(B)]
            E_sb = [sb.tile([NM, 2 * TB], BF16, tag=f"e{b}", name=f"e{b}") for b in range(B)]
            RS = [sb.tile([NM, 2], F32, tag=f"rs{b}", name=f"rs{b}") for b in range(B)]
            RC = [sb.tile([NM, 2], F32, tag=f"rcol{b}", name=f"rcol{b}") for b in range(B)]
            RS1 = [sb.tile([NM, 2], F32, tag=f"rs1{b}", name=f"rs1{b}") for b in range(B)]
            RREP = [sb.tile([NM, 128], BF16, tag=f"rrep{b}", name=f"rrep{b}") for b in range(B)]
            W_ps = [ps.tile([128, 512], F32, tag=f"wps{b}", name=f"wps{b}") for b in range(B)]

            def scores_sm(b):
                for h in range(2):
                    sl = slice(h * HD, (h + 1) * HD)
                    nc.tensor.matmul(S_ps[b][:, h * TB:(h + 1) * TB],
                                     q0r[sl, b * NM:(b + 1) * NM],
                                     k_sb[sl, b * TB:(b + 1) * TB],
                                     start=True, stop=True)
                    nc.vector.tensor_mul(SM[b][:, h * TB:(h + 1) * TB],
                                         S_ps[b][:, h * TB:(h + 1) * TB],
                                         msb[:, b * TB:(b + 1) * TB])

            def exps(b):
                # pad col of sm is 0 -> exp=1; row sum corrected by -1
                for h in range(2):
                    nc.scalar.activation(E_sb[b][:, h * TB:(h + 1) * TB],
                                         SM[b][:, h * TB:(h + 1) * TB], AF.Exp,
                                         accum_out=RS[b][:, h:h + 1])

            def normalize(b):
                nc.vector.tensor_scalar_add(RS1[b][:], RS[b][:], -1.0)
                nc.vector.reciprocal(RC[b][:], RS1[b][:])
                for h in range(2):
                    nc.vector.tensor_scalar_mul(
                        RREP[b][:, h * HD:(h + 1) * HD], ones[0:NM, 0:HD],
                        RC[b][:, h:h + 1])

            def weighted_v(b):
                # one matmul per head, output at partition base h*64 so both
                # heads' weight rows line up with V's channel layout
                for h in range(2):
                    nc.tensor.matmul(W_ps[b][h * HD:(h + 1) * HD, 0:TB],
                                     RREP[b][:, h * HD:(h + 1) * HD],
                                     E_sb[b][:, h * TB:(h + 1) * TB],
                                     start=True, stop=True)

            def attn_out(b):
                t = sb.tile([128, L], BF16, tag=f"t{b}", name=f"t{b}")
                # attn0[c] = sum_d w[d] * v[c, d] over the 197 real cols
                nc.vector.scalar_tensor_tensor(
                    t[:], W_ps[b][:, 0:L], 1.0,
                    v_sb[:, b * TB: b * TB + L],
                    op0=ALU.mult, op1=ALU.mult,
                    accum_out=A0[:, b:b + 1])

            scores_sm(0)
            exps(0)
            scores_sm(1)
            exps(1)
            normalize(0)
            nc.vector.tensor_scalar_add(v_sb[:], V_ps[:, 0:2 * TB], vb_col)
            weighted_v(0)
            normalize(1)
            attn_out(0)
            weighted_v(1)
            attn_out(1)

            # ---- c-proj, flipped: out[2c+b, k] = sum_e A0[e,b] cwt[e, 512c+k]
            A0r = sb.tile([128, 2], BF16, tag="a0r")
            nc.gpsimd.tensor_copy(A0r[:], A0[:])
            for c in range(2):
                nc.tensor.matmul(o_ps[32 * c:32 * c + 2, 0:512],
                                 A0r[:],
                                 BL[:, C_CWT + 512 * c: C_CWT + 512 * (c + 1)],
                                 start=False, stop=True, skip_group_check=True)
            o_sb = sb.tile([34, 512], BF16, tag="osb")
            nc.vector.tensor_copy(o_sb[0:2, :], o_ps[0:2, :])
            nc.vector.tensor_copy(o_sb[32:34, :], o_ps[32:34, :])
            nc.scalar.dma_start(out_ap[0:2, :], o_sb[0:2, :])
            nc.sync.dma_start(out_ap[32:34, :], o_sb[32:34, :])

    nc.compile()
    return nc


def _get_nc():
    if "nc" not in _STATE:
        _STATE["nc"] = _build()
    return _STATE["nc"]


def _make_in_maps(inputs):
    """Host-side packing: pure layout/dtype movement into one blob per core."""
    import ml_dtypes

    x = np.asarray(inputs["x"], np.float32).reshape(B, E, S)
    mask_feature = np.asarray(inputs["mask_feature"], np.float32)
    pos_t = np.ascontiguousarray(np.asarray(inputs["pos_emb"], np.float32).T)
    q_w = np.asarray(inputs["q_w"], np.float32)
    q_b = np.asarray(inputs["q_b"], np.float32)
    k_w = np.asarray(inputs["k_w"], np.float32)
    k_b = np.asarray(inputs["k_b"], np.float32)
    v_w = np.asarray(inputs["v_w"], np.float32)
    v_b = np.asarray(inputs["v_b"], np.float32)
    c_w = np.asarray(inputs["c_w"], np.float32)
    c_b = np.asarray(inputs["c_b"], np.float32)

    mask12 = mask_feature[:, :, ::8, ::8].reshape(B, NM, S)

    in_maps = []
    for c in range(NCORES):
        ch = slice(c * 128, (c + 1) * 128)
        blob = np.zeros((128, NCOLS), np.float32)
        for et in range(NET):
            base = et * ETCOLS
            eslc = slice(et * 128, (et + 1) * 128)
            blob[:, base:base + 128] = k_w[ch, eslc].T
            blob[:, base + 128:base + 256] = v_w[ch, eslc].T
            blob[:, base + 256:base + 384] = q_w[ch, eslc].T
            for b in range(B):
                blob[:, base + 384 + b * TB + 1: base + 384 + b * TB + L] = \
                    x[b, eslc]
            blob[:, base + 384 + 2 * TB: base + 384 + 2 * TB + L] = \
                pos_t[eslc]
        blob[:, C_CWT:C_CWT + E] = c_w[:, ch].T
        blob[:, C_KB] = k_b[ch]
        blob[:, C_VB] = v_b[ch]
        blob[:, C_QB] = q_b[ch]
        blob[0:NM, C_MASK:C_MASK + S] = mask12[0]
        blob[0:NM, C_MASK + S:C_MASK + 2 * S] = mask12[1]
        for et in range(NET):
            eslc = slice(et * 128, (et + 1) * 128)
            blob[:, C_P0 + 2 * et] = pos_t[eslc, 0]
            blob[:, C_P0 + 2 * et + 1] = pos_t[eslc, 0]
        # c_b rows 0/1 = c_b[0:512] (b0/b1), rows 32/33 = c_b[512:1024]
        cb = np.zeros((34, 512), np.float32)
        cb[0:2] = c_b[0:512]
        cb[32:34] = c_b[512:1024]
        in_maps.append({"blob": blob.astype(ml_dtypes.bfloat16),
                        "cb": cb.astype(ml_dtypes.bfloat16)})
    return in_maps


def _unshard(parts):
    """Sum per-core partial outputs [34, 512] -> [B, E]."""
    R = np.zeros((34, 512), np.float64)
    for p in parts:
        R += np.asarray(p, np.float32)
    return np.ascontiguousarray(
        np.concatenate([R[0:2], R[32:34]], axis=1)).astype(np.float32)


def kernel(**inputs):
    in_maps = _make_in_maps(inputs)

    from concourse.bass_utils import run_bass_kernel_spmd

    nc = _get_nc()
    trace = bool(int(os.environ.get("KERNEL_TRACE", "0")))
    if trace:
        try:
            import ntff_hook
            ntff_hook.install()
        except Exception:
            pass
    res = run_bass_kernel_spmd(nc, in_maps, list(range(NCORES)), trace=trace)
    _STATE["last_exec_ns"] = res.exec_time_ns
    _STATE["last_results"] = res
    return _unshard([res.results[c]["out"] for c in range(NCORES)])
